# revision 19
# baseline (speedup 1.0000x reference)
"""DFFN Trainium2 kernel: 1x1 conv -> 2x2 FFT gate -> 3x3 depthwise conv -> gelu-gate -> 1x1 conv.

Data-parallel over batch: 8 NeuronCores, one 192x128x128 image each.

v3 design:
- All matmul operands bf16 (fp32 streams the PE at 2 cycles/column; bf16 at 1).
  PSUM accumulation stays fp32; the final out-proj eviction + output stay fp32.
- The forward 2x2 Hadamard butterfly over x is applied on the HOST (it is a
  linear transform of the input); the device loads pre-transformed planes.
- Hidden (hp) tiles pack partitions 0-63 with the gelu-branch channels (x1,
  conv inputs 0..254) and 64-127 with the gate-branch channels (x2, inputs
  255..509) of the SAME output chunk, via a host-side column permutation of
  w_in.  The depthwise conv runs as row-tiled (64x128) matmul pairs on PE
  tiles T0/T8 which stream CONCURRENTLY (~120 ns/MM measured), and each
  pair's two PSUM banks feed the gelu-gate immediately.
- Two-band software pipeline: the hidden tensor for band b+2 is computed
  during band b, so the dwconv never waits on in-proj/butterfly results and
  the PE stays busy end-to-end (no HAM re-throttle).
- No halo recompute: each band computes exactly its 8 patch rows; the one
  hidden halo row needed on each side is copied from the neighbor band's
  tile with a tiny SBUF->SBUF DMA (image edges are zeroed).
- Per band, all 288 row-tiled dwconv matmuls run in one block, then all
  full-array matmuls (out-proj + band b+2 in-proj), so the PE tiling mode
  only switches twice per band.
"""
import numpy as np
from contextlib import ExitStack

import ml_dtypes

import concourse.bass as bass
import concourse.bacc as bacc
import concourse.tile as tile
from concourse import mybir
from concourse.bass_utils import run_bass_kernel_spmd

F32 = mybir.dt.float32
BF16 = mybir.dt.bfloat16
BF16NP = ml_dtypes.bfloat16

DIM, HID = 192, 510
H = W = 128
NSLOT = 512
NB = 8          # row bands (16 image rows each)
PADW = 130      # padded row width
NCORES = 8
OW = [128, 128, 128, 126]   # valid widths of the 4 output-chunk pairs
NPATCH = 512    # 8 patch rows x 64 patch cols per band

H4 = np.array([[1, 1, 1, 1],
               [1, -1, 1, -1],
               [1, 1, -1, -1],
               [1, -1, -1, 1]], np.float32)

HID_OF_SLOT = np.full(NSLOT, -1, np.int64)
HID_OF_SLOT[0:255] = np.arange(0, 255)
HID_OF_SLOT[256:511] = np.arange(255, 510)
VALID_SLOT = HID_OF_SLOT >= 0

# pair-packed slot order: block a holds x1 slots 64a..64a+63 then x2 slots
# 256+64a..256+64a+63, so in-proj chunk a lands in the hp-tile partition
# layout the row-tiled dwconv wants.
PAIR_PERM = np.zeros(NSLOT, np.int64)
for _a in range(4):
    PAIR_PERM[128 * _a:128 * _a + 64] = 64 * _a + np.arange(64)
    PAIR_PERM[128 * _a + 64:128 * (_a + 1)] = 256 + 64 * _a + np.arange(64)


def build_module(act="gelu"):
    act_fn = {"gelu": mybir.ActivationFunctionType.Gelu,
              "identity": mybir.ActivationFunctionType.Identity}[act]
    nc = bacc.Bacc()
    xp_d = nc.declare_dram_parameter("xp", [DIM, 4 * 4096], BF16, isOutput=False)
    win_d = nc.declare_dram_parameter("w_in4", [2, 96, 4 * NSLOT], BF16, isOutput=False)
    dws_d = nc.declare_dram_parameter("dw_stat", [128, 36 * 128], BF16, isOutput=False)
    wout_d = nc.declare_dram_parameter("w_outT", [128, 4 * 192], BF16, isOutput=False)
    out_d = nc.declare_dram_parameter("out", [DIM, H * W], F32, isOutput=True)

    with tile.TileContext(nc) as tc, ExitStack() as ctx:
        wpool = ctx.enter_context(tc.tile_pool(name="weights", bufs=1))
        xpool = ctx.enter_context(tc.tile_pool(name="xin", bufs=3))
        htpool = ctx.enter_context(tc.tile_pool(name="ht", bufs=2))
        hqpool = ctx.enter_context(tc.tile_pool(name="hstage", bufs=2))
        hppool = ctx.enter_context(tc.tile_pool(name="hpad", bufs=3))
        t1pool = ctx.enter_context(tc.tile_pool(name="t1", bufs=2))
        gpool = ctx.enter_context(tc.tile_pool(name="g", bufs=16))
        opool = ctx.enter_context(tc.tile_pool(name="osb", bufs=4))
        # in-proj and out-proj both run in the full-array phase, so they share
        # one 4-bank PSUM pool: each bank gets a multi-microsecond reuse
        # window and evictions never pace the matmul stream.
        mx_ps = ctx.enter_context(tc.tile_pool(name="mx_ps", bufs=4, space=bass.MemorySpace.PSUM))
        dw_ps = ctx.enter_context(tc.tile_pool(name="dw_ps", bufs=4, space=bass.MemorySpace.PSUM))

        # ---- weights, loaded once (win first: it gates the first in-proj)
        win_t = []
        for kc in range(2):
            wt = wpool.tile([96, 4 * NSLOT], BF16, tag=f"win{kc}")
            for k in range(4):
                nc.sync.dma_start(wt[:, k * NSLOT:(k + 1) * NSLOT],
                                  win_d[kc][:, k * NSLOT:(k + 1) * NSLOT])
            win_t.append(wt)
        zt = wpool.tile([128, 260], BF16, tag="zero")
        nc.vector.memset(zt[:, :], 0.0)

        def emit_wload2():
            dws_t = wpool.tile([128, 36 * 128], BF16, tag="dws")
            for q in range(4):
                nc.sync.dma_start(dws_t[:, q * 1152:(q + 1) * 1152],
                                  dws_d[:, q * 1152:(q + 1) * 1152])
            wout_t = wpool.tile([128, 4 * 192], BF16, tag="wout")
            nc.sync.dma_start(wout_t[:, :], wout_d[:, :])
            return dws_t, wout_t

        def emit_load(b):
            """DMA the pre-butterflied x planes for one band (8 patch rows)."""
            xt_k = []
            for kc in range(2):
                xt_b = xpool.tile([96, 4 * NPATCH], BF16, tag=f"xt{kc}")
                for k in range(4):
                    nc.sync.dma_start(
                        xt_b[:, k * NPATCH:(k + 1) * NPATCH],
                        xp_d[96 * kc:96 * kc + 96,
                             k * 4096 + 8 * b * 64:k * 4096 + (8 * b + 8) * 64],
                    )
                xt_k.append(xt_b)
            return xt_k

        def emit_pair(b, a, xt_k):
            """in-proj for pair-chunk a (x1 slots 64a.., x2 slots 256+64a..),
            evict (ACT), inverse butterfly into rows 2..17 of the hp tile."""
            eng = nc.vector if a < 2 else nc.gpsimd
            ht_t = htpool.tile([128, 4 * NPATCH], BF16, tag="ht")
            for k in range(4):
                ps = mx_ps.tile([128, 512], F32, tag="ps")
                for kc in range(2):
                    nc.tensor.matmul(
                        ps[:, :],
                        win_t[kc][:, k * NSLOT + 128 * a:k * NSLOT + 128 * (a + 1)],
                        xt_k[kc][:, k * NPATCH:(k + 1) * NPATCH],
                        start=(kc == 0), stop=(kc == 1),
                    )
                nc.scalar.copy(ht_t[:, k * NPATCH:(k + 1) * NPATCH], ps[:, :])

            hp_t = hppool.tile([128, 20 * PADW], BF16, tag=f"hp{a}")
            hp3 = hp_t[:, :].rearrange("p (ly c) -> p ly c", ly=20, c=130)
            eng.tensor_copy(hp3[:, 1:19, 0], zt[:, 0:18])
            eng.tensor_copy(hp3[:, 1:19, 129], zt[:, 0:18])

            hr = ht_t[:, :].rearrange("p (kp k2 n) -> p kp k2 n", kp=2, k2=2, n=NPATCH)
            h02 = hr[:, :, 0, :]                # planes 0,2: [128, 2, NPATCH]
            h13 = hr[:, :, 1, :]
            squ = hqpool.tile([128, 2 * NPATCH], BF16, tag="squ")
            tqv = hqpool.tile([128, 2 * NPATCH], BF16, tag="tqv")
            squ_w = squ[:, :].rearrange("p (k n) -> p k n", k=2)
            tqv_w = tqv[:, :].rearrange("p (k n) -> p k n", k=2)
            eng.tensor_add(squ_w, h02, h13)             # s | u
            eng.tensor_sub(tqv_w, h02, h13)             # t | v
            s_ap = squ_w[:, 0, :].rearrange("p (lp px) -> p lp px", lp=8, px=64)
            u_ap = squ_w[:, 1, :].rearrange("p (lp px) -> p lp px", lp=8, px=64)
            t_ap = tqv_w[:, 0, :].rearrange("p (lp px) -> p lp px", lp=8, px=64)
            v_ap = tqv_w[:, 1, :].rearrange("p (lp px) -> p lp px", lp=8, px=64)

            def wr(iy, ix):
                return hp3[:, 2 + iy:2 + iy + 16:2, 1 + ix:1 + ix + 127:2]
            eng.tensor_add(wr(0, 0), s_ap, u_ap)
            eng.tensor_add(wr(0, 1), t_ap, v_ap)
            eng.tensor_sub(wr(1, 0), s_ap, u_ap)
            eng.tensor_sub(wr(1, 1), t_ap, v_ap)
            return hp_t

        def emit_halo(hp_prev, hp_next):
            """fill hidden halo rows: row 18 of prev band <- row 3 of next
            band; row 1 of next band <- row 17 of prev band."""
            for a in range(4):
                nc.sync.dma_start(hp_prev[a][:, 18 * PADW:19 * PADW],
                                  hp_next[a][:, 2 * PADW:3 * PADW])
                nc.sync.dma_start(hp_next[a][:, 1 * PADW:2 * PADW],
                                  hp_prev[a][:, 17 * PADW:18 * PADW])

        def emit_edge_zero(hp_b, top):
            for a in range(4):
                if top:
                    nc.gpsimd.tensor_copy(hp_b[a][:, 1 * PADW:2 * PADW], zt[:, 0:130])
                else:
                    nc.gpsimd.tensor_copy(hp_b[a][:, 18 * PADW:19 * PADW], zt[:, 0:130])

        def emit_dw_gate(b, tt, hp_cur):
            """row-tiled dwconv (T0: x1 chunk, T8: x2 chunk) + gelu-gate."""
            g_a = []
            for a in range(4):
                owa = OW[a]
                ps1 = dw_ps.tile([128, 512], F32, tag="dw")
                ps2 = dw_ps.tile([128, 512], F32, tag="dw")
                hp3 = hp_cur[a][:, :].rearrange("p (ly c) -> p ly c", ly=20, c=130)
                for t in range(9):
                    dyi, dxi = t // 3, t % 3
                    r0 = 1 + 4 * tt + dyi
                    lcol = (a * 9 + t) * 128
                    nc.tensor.matmul(
                        ps1[0:owa, :],
                        dws_t[0:64, lcol:lcol + owa],
                        hp3[0:64, r0:r0 + 4, dxi:dxi + 128],
                        start=(t == 0), stop=(t == 8))
                    nc.tensor.matmul(
                        ps2[0:owa, :],
                        dws_t[64:128, lcol:lcol + owa],
                        hp3[64:128, r0:r0 + 4, dxi:dxi + 128],
                        start=(t == 0), stop=(t == 8))
                t1 = t1pool.tile([128, 512], F32, tag="t1")
                g_t = gpool.tile([128, 512], BF16, tag="g")
                with tc.high_priority():
                    # evacuate the pair's PSUM banks ASAP so the next dwconv
                    # pair never stalls (and no full-mode MM sneaks in)
                    nc.scalar.activation(t1[0:owa, :], ps1[0:owa, :], act_fn)
                    nc.vector.tensor_mul(g_t[0:owa, :], t1[0:owa, :], ps2[0:owa, :])
                g_a.append(g_t)
            return g_a

        def emit_outproj(b, tt, g_a):
            osb = opool.tile([96, 1024], F32, tag="osb")
            off = b * 2048 + tt * 512
            for mo in range(2):
                ops_t = mx_ps.tile([96, 512], F32, tag="ps")
                for a in range(4):
                    kw = OW[a]
                    nc.tensor.matmul(
                        ops_t[:, :],
                        wout_t[0:kw, a * 192 + 96 * mo:a * 192 + 96 * (mo + 1)],
                        g_a[a][0:kw, :],
                        start=(a == 0), stop=(a == 3),
                    )
                nc.scalar.copy(osb[:, mo * 512:mo * 512 + 512], ops_t[:, :])
                nc.sync.dma_start(
                    out_d[96 * mo:96 * mo + 96, off:off + 512],
                    osb[:, mo * 512:mo * 512 + 512])

        # ---- two-band-deep pipeline; x-plane DMAs prefetched a band earlier
        # still so the in-proj matmuls never wait on them.
        xt = {0: emit_load(0)}
        hp = {0: [emit_pair(0, a, xt[0]) for a in range(4)]}
        dws_t, wout_t = emit_wload2()
        xt[1] = emit_load(1)
        hp[1] = [emit_pair(1, a, xt[1]) for a in range(4)]
        xt[2] = emit_load(2)
        emit_edge_zero(hp[0], top=True)
        emit_halo(hp[0], hp[1])
        for b in range(NB):
            gs = [emit_dw_gate(b, tt, hp[b]) for tt in range(4)]
            if b + 3 < NB:
                xt[b + 3] = emit_load(b + 3)
            for tt in range(4):
                emit_outproj(b, tt, gs[tt])
                if b + 2 < NB:
                    hp.setdefault(b + 2, []).append(emit_pair(b + 2, tt, xt[b + 2]))
            if b + 2 < NB:
                emit_halo(hp[b + 1], hp[b + 2])
                del xt[b + 2]
            elif b + 2 == NB:
                emit_edge_zero(hp[NB - 1], top=False)
            del hp[b]
    nc.finalize()
    return nc


def prep_weights(w_in, w_dw, fft_w, w_out):
    w_in2 = np.asarray(w_in)[:, :, 0, 0].astype(np.float32)        # [510, 192]
    w_dw2 = np.asarray(w_dw)[:, 0].reshape(2 * HID, 9).astype(np.float32)
    wf = np.asarray(fft_w)[:, 0, 0].reshape(HID, 4).astype(np.float32)
    w_out2 = np.asarray(w_out)[:, :, 0, 0].astype(np.float32)      # [192, 510]

    wslot = np.zeros((NSLOT, DIM), np.float32)
    fslot = np.zeros((NSLOT, 4), np.float32)
    wslot[VALID_SLOT] = w_in2[HID_OF_SLOT[VALID_SLOT]]
    fslot[VALID_SLOT] = wf[HID_OF_SLOT[VALID_SLOT]] * 0.25
    wperm = wslot[PAIR_PERM]
    fperm = fslot[PAIR_PERM]
    win4 = np.zeros((2, 96, 4 * NSLOT), np.float32)
    for kc in range(2):
        for k in range(4):
            win4[kc, :, k * NSLOT:(k + 1) * NSLOT] = (
                wperm[:, 96 * kc:96 * (kc + 1)] * fperm[:, k:k + 1]).T

    # dwconv stationary: parts 0-63 = x1 (output o=128a+m reads channel
    # 64a+m//2 at part m//2), parts 64-127 = x2 (output 510+128a+m reads
    # channel 255+64a+m//2 at part 64+m//2).
    dws = np.zeros((128, 36 * 128), np.float32)
    for a in range(4):
        for t in range(9):
            col = (a * 9 + t) * 128
            for m in range(OW[a]):
                dws[m // 2, col + m] = w_dw2[128 * a + m, t]
                dws[64 + m // 2, col + m] = w_dw2[510 + 128 * a + m, t]

    woutT = np.zeros((128, 4 * 192), np.float32)
    for a in range(4):
        kw = OW[a]
        woutT[0:kw, a * 192:(a + 1) * 192] = w_out2[:, 128 * a:128 * a + kw].T
    return (win4.astype(BF16NP), dws.astype(BF16NP), woutT.astype(BF16NP))


def prep_x_planes(x1img):
    """Forward 2x2 Hadamard butterfly on the host -> [DIM, 4*4096] bf16."""
    xr = np.asarray(x1img, np.float32).reshape(DIM, 64, 2, 64, 2)
    q = np.stack([xr[:, :, 0, :, 0], xr[:, :, 0, :, 1],
                  xr[:, :, 1, :, 0], xr[:, :, 1, :, 1]], 1)   # [DIM,4,64,64]
    planes = np.einsum('kq,cqyx->ckyx', H4, q)
    return planes.reshape(DIM, 4 * 4096).astype(BF16NP)


_NC = None


def kernel(x, w_in, w_dw, fft_w, w_out):
    global _NC
    if _NC is None:
        _NC = build_module()
    win4, dws, woutT = prep_weights(w_in, w_dw, fft_w, w_out)
    x = np.ascontiguousarray(np.asarray(x), dtype=np.float32)
    in_maps = [
        {"xp": prep_x_planes(x[i]), "w_in4": win4, "dw_stat": dws, "w_outT": woutT}
        for i in range(NCORES)
    ]
    res = run_bass_kernel_spmd(_NC, in_maps, list(range(NCORES)))
    out = np.stack([res.results[i]["out"].reshape(DIM, H, W) for i in range(NCORES)])
    return out.astype(np.float32)


# revision 21
# speedup vs baseline: 1.0009x; 1.0009x over previous
"""DFFN Trainium2 kernel: 1x1 conv -> 2x2 FFT gate -> 3x3 depthwise conv -> gelu-gate -> 1x1 conv.

Data-parallel over batch: 8 NeuronCores, one 192x128x128 image each.

v3 design:
- All matmul operands bf16 (fp32 streams the PE at 2 cycles/column; bf16 at 1).
  PSUM accumulation stays fp32; the final out-proj eviction + output stay fp32.
- The forward 2x2 Hadamard butterfly over x is applied on the HOST (it is a
  linear transform of the input); the device loads pre-transformed planes.
- Hidden (hp) tiles pack partitions 0-63 with the gelu-branch channels (x1,
  conv inputs 0..254) and 64-127 with the gate-branch channels (x2, inputs
  255..509) of the SAME output chunk, via a host-side column permutation of
  w_in.  The depthwise conv runs as row-tiled (64x128) matmul pairs on PE
  tiles T0/T8 which stream CONCURRENTLY (~120 ns/MM measured), and each
  pair's two PSUM banks feed the gelu-gate immediately.
- Two-band software pipeline: the hidden tensor for band b+2 is computed
  during band b, so the dwconv never waits on in-proj/butterfly results and
  the PE stays busy end-to-end (no HAM re-throttle).
- No halo recompute: each band computes exactly its 8 patch rows; the one
  hidden halo row needed on each side is copied from the neighbor band's
  tile with a tiny SBUF->SBUF DMA (image edges are zeroed).
- Per band, all 288 row-tiled dwconv matmuls run in one block, then all
  full-array matmuls (out-proj + band b+2 in-proj), so the PE tiling mode
  only switches twice per band.
"""
import numpy as np
from contextlib import ExitStack

import ml_dtypes

import concourse.bass as bass
import concourse.bacc as bacc
import concourse.tile as tile
from concourse import mybir
from concourse.bass_utils import run_bass_kernel_spmd

F32 = mybir.dt.float32
BF16 = mybir.dt.bfloat16
BF16NP = ml_dtypes.bfloat16

DIM, HID = 192, 510
H = W = 128
NSLOT = 512
NB = 8          # row bands (16 image rows each)
PADW = 130      # padded row width
NCORES = 8
OW = [128, 128, 128, 126]   # valid widths of the 4 output-chunk pairs
NPATCH = 512    # 8 patch rows x 64 patch cols per band

H4 = np.array([[1, 1, 1, 1],
               [1, -1, 1, -1],
               [1, 1, -1, -1],
               [1, -1, -1, 1]], np.float32)

HID_OF_SLOT = np.full(NSLOT, -1, np.int64)
HID_OF_SLOT[0:255] = np.arange(0, 255)
HID_OF_SLOT[256:511] = np.arange(255, 510)
VALID_SLOT = HID_OF_SLOT >= 0

# pair-packed slot order: block a holds x1 slots 64a..64a+63 then x2 slots
# 256+64a..256+64a+63, so in-proj chunk a lands in the hp-tile partition
# layout the row-tiled dwconv wants.
PAIR_PERM = np.zeros(NSLOT, np.int64)
for _a in range(4):
    PAIR_PERM[128 * _a:128 * _a + 64] = 64 * _a + np.arange(64)
    PAIR_PERM[128 * _a + 64:128 * (_a + 1)] = 256 + 64 * _a + np.arange(64)


def build_module(act="gelu"):
    act_fn = {"gelu": mybir.ActivationFunctionType.Gelu,
              "identity": mybir.ActivationFunctionType.Identity}[act]
    nc = bacc.Bacc()
    xp_d = nc.declare_dram_parameter("xp", [DIM, 4 * 4096], BF16, isOutput=False)
    win_d = nc.declare_dram_parameter("w_in4", [2, 96, 4 * NSLOT], BF16, isOutput=False)
    dws_d = nc.declare_dram_parameter("dw_stat", [128, 36 * 128], BF16, isOutput=False)
    wout_d = nc.declare_dram_parameter("w_outT", [128, 4 * 192], BF16, isOutput=False)
    out_d = nc.declare_dram_parameter("out", [DIM, H * W], F32, isOutput=True)

    with tile.TileContext(nc) as tc, ExitStack() as ctx:
        wpool = ctx.enter_context(tc.tile_pool(name="weights", bufs=1))
        xpool = ctx.enter_context(tc.tile_pool(name="xin", bufs=2))
        htpool = ctx.enter_context(tc.tile_pool(name="ht", bufs=2))
        hqpool = ctx.enter_context(tc.tile_pool(name="hstage", bufs=2))
        hppool = ctx.enter_context(tc.tile_pool(name="hpad", bufs=3))
        t1pool = ctx.enter_context(tc.tile_pool(name="t1", bufs=2))
        gpool = ctx.enter_context(tc.tile_pool(name="g", bufs=16))
        opool = ctx.enter_context(tc.tile_pool(name="osb", bufs=4))
        # in-proj and out-proj both run in the full-array phase, so they share
        # one 4-bank PSUM pool: each bank gets a multi-microsecond reuse
        # window and evictions never pace the matmul stream.
        mx_ps = ctx.enter_context(tc.tile_pool(name="mx_ps", bufs=4, space=bass.MemorySpace.PSUM))
        dw_ps = ctx.enter_context(tc.tile_pool(name="dw_ps", bufs=4, space=bass.MemorySpace.PSUM))

        # ---- weights, loaded once (win first: it gates the first in-proj)
        win_t = []
        for kc in range(2):
            wt = wpool.tile([96, 4 * NSLOT], BF16, tag=f"win{kc}")
            for k in range(4):
                nc.sync.dma_start(wt[:, k * NSLOT:(k + 1) * NSLOT],
                                  win_d[kc][:, k * NSLOT:(k + 1) * NSLOT])
            win_t.append(wt)
        zt = wpool.tile([128, 260], BF16, tag="zero")
        nc.vector.memset(zt[:, :], 0.0)

        def emit_wload2():
            dws_t = wpool.tile([128, 36 * 128], BF16, tag="dws")
            for q in range(4):
                nc.sync.dma_start(dws_t[:, q * 1152:(q + 1) * 1152],
                                  dws_d[:, q * 1152:(q + 1) * 1152])
            wout_t = wpool.tile([128, 4 * 192], BF16, tag="wout")
            nc.sync.dma_start(wout_t[:, :], wout_d[:, :])
            return dws_t, wout_t

        def emit_load(b):
            """DMA the pre-butterflied x planes for one band (8 patch rows)."""
            xt_k = []
            for kc in range(2):
                xt_b = xpool.tile([96, 4 * NPATCH], BF16, tag=f"xt{kc}")
                for k in range(4):
                    nc.sync.dma_start(
                        xt_b[:, k * NPATCH:(k + 1) * NPATCH],
                        xp_d[96 * kc:96 * kc + 96,
                             k * 4096 + 8 * b * 64:k * 4096 + (8 * b + 8) * 64],
                    )
                xt_k.append(xt_b)
            return xt_k

        def emit_pair(b, a, xt_k):
            """in-proj for pair-chunk a (x1 slots 64a.., x2 slots 256+64a..),
            evict (ACT), inverse butterfly into rows 2..17 of the hp tile."""
            eng = nc.vector if a < 2 else nc.gpsimd
            ht_t = htpool.tile([128, 4 * NPATCH], BF16, tag="ht")
            for k in range(4):
                ps = mx_ps.tile([128, 512], F32, tag="ps")
                for kc in range(2):
                    nc.tensor.matmul(
                        ps[:, :],
                        win_t[kc][:, k * NSLOT + 128 * a:k * NSLOT + 128 * (a + 1)],
                        xt_k[kc][:, k * NPATCH:(k + 1) * NPATCH],
                        start=(kc == 0), stop=(kc == 1),
                    )
                nc.scalar.copy(ht_t[:, k * NPATCH:(k + 1) * NPATCH], ps[:, :])

            hp_t = hppool.tile([128, 20 * PADW], BF16, tag=f"hp{a}")
            hp3 = hp_t[:, :].rearrange("p (ly c) -> p ly c", ly=20, c=130)
            eng.tensor_copy(hp3[:, 1:19, 0], zt[:, 0:18])
            eng.tensor_copy(hp3[:, 1:19, 129], zt[:, 0:18])

            hr = ht_t[:, :].rearrange("p (kp k2 n) -> p kp k2 n", kp=2, k2=2, n=NPATCH)
            h02 = hr[:, :, 0, :]                # planes 0,2: [128, 2, NPATCH]
            h13 = hr[:, :, 1, :]
            squ = hqpool.tile([128, 2 * NPATCH], BF16, tag="squ")
            tqv = hqpool.tile([128, 2 * NPATCH], BF16, tag="tqv")
            squ_w = squ[:, :].rearrange("p (k n) -> p k n", k=2)
            tqv_w = tqv[:, :].rearrange("p (k n) -> p k n", k=2)
            eng.tensor_add(squ_w, h02, h13)             # s | u
            eng.tensor_sub(tqv_w, h02, h13)             # t | v
            s_ap = squ_w[:, 0, :].rearrange("p (lp px) -> p lp px", lp=8, px=64)
            u_ap = squ_w[:, 1, :].rearrange("p (lp px) -> p lp px", lp=8, px=64)
            t_ap = tqv_w[:, 0, :].rearrange("p (lp px) -> p lp px", lp=8, px=64)
            v_ap = tqv_w[:, 1, :].rearrange("p (lp px) -> p lp px", lp=8, px=64)

            def wr(iy, ix):
                return hp3[:, 2 + iy:2 + iy + 16:2, 1 + ix:1 + ix + 127:2]
            eng.tensor_add(wr(0, 0), s_ap, u_ap)
            eng.tensor_add(wr(0, 1), t_ap, v_ap)
            eng.tensor_sub(wr(1, 0), s_ap, u_ap)
            eng.tensor_sub(wr(1, 1), t_ap, v_ap)
            return hp_t

        def emit_halo(hp_prev, hp_next):
            """fill hidden halo rows: row 18 of prev band <- row 3 of next
            band; row 1 of next band <- row 17 of prev band."""
            for a in range(4):
                nc.sync.dma_start(hp_prev[a][:, 18 * PADW:19 * PADW],
                                  hp_next[a][:, 2 * PADW:3 * PADW])
                nc.sync.dma_start(hp_next[a][:, 1 * PADW:2 * PADW],
                                  hp_prev[a][:, 17 * PADW:18 * PADW])

        def emit_edge_zero(hp_b, top):
            for a in range(4):
                if top:
                    nc.gpsimd.tensor_copy(hp_b[a][:, 1 * PADW:2 * PADW], zt[:, 0:130])
                else:
                    nc.gpsimd.tensor_copy(hp_b[a][:, 18 * PADW:19 * PADW], zt[:, 0:130])

        def emit_dw_gate(b, tt, hp_cur):
            """row-tiled dwconv (T0: x1 chunk, T8: x2 chunk) + gelu-gate."""
            g_a = []
            for a in range(4):
                owa = OW[a]
                ps1 = dw_ps.tile([128, 512], F32, tag="dw")
                ps2 = dw_ps.tile([128, 512], F32, tag="dw")
                hp3 = hp_cur[a][:, :].rearrange("p (ly c) -> p ly c", ly=20, c=130)
                for t in range(9):
                    dyi, dxi = t // 3, t % 3
                    r0 = 1 + 4 * tt + dyi
                    lcol = (a * 9 + t) * 128
                    nc.tensor.matmul(
                        ps1[0:owa, :],
                        dws_t[0:64, lcol:lcol + owa],
                        hp3[0:64, r0:r0 + 4, dxi:dxi + 128],
                        start=(t == 0), stop=(t == 8))
                    nc.tensor.matmul(
                        ps2[0:owa, :],
                        dws_t[64:128, lcol:lcol + owa],
                        hp3[64:128, r0:r0 + 4, dxi:dxi + 128],
                        start=(t == 0), stop=(t == 8))
                t1 = t1pool.tile([128, 512], F32, tag="t1")
                g_t = gpool.tile([128, 512], BF16, tag="g")
                with tc.high_priority():
                    # evacuate the pair's PSUM banks ASAP so the next dwconv
                    # pair never stalls (and no full-mode MM sneaks in)
                    nc.scalar.activation(t1[0:owa, :], ps1[0:owa, :], act_fn)
                    nc.vector.tensor_mul(g_t[0:owa, :], t1[0:owa, :], ps2[0:owa, :])
                g_a.append(g_t)
            return g_a

        def emit_outproj(b, tt, g_a):
            osb = opool.tile([96, 1024], F32, tag="osb")
            off = b * 2048 + tt * 512
            for mo in range(2):
                ops_t = mx_ps.tile([96, 512], F32, tag="ps")
                for a in range(4):
                    kw = OW[a]
                    nc.tensor.matmul(
                        ops_t[:, :],
                        wout_t[0:kw, a * 192 + 96 * mo:a * 192 + 96 * (mo + 1)],
                        g_a[a][0:kw, :],
                        start=(a == 0), stop=(a == 3),
                    )
                nc.scalar.copy(osb[:, mo * 512:mo * 512 + 512], ops_t[:, :])
                nc.sync.dma_start(
                    out_d[96 * mo:96 * mo + 96, off:off + 512],
                    osb[:, mo * 512:mo * 512 + 512])

        # ---- two-band-deep pipeline.
        xt = emit_load(0)
        hp = {0: [emit_pair(0, a, xt) for a in range(4)]}
        dws_t, wout_t = emit_wload2()
        xt = emit_load(1)
        hp[1] = [emit_pair(1, a, xt) for a in range(4)]
        emit_edge_zero(hp[0], top=True)
        emit_halo(hp[0], hp[1])
        for b in range(NB):
            gs = [emit_dw_gate(b, tt, hp[b]) for tt in range(4)]
            xt_nxt = emit_load(b + 2) if b + 2 < NB else None
            for tt in range(4):
                emit_outproj(b, tt, gs[tt])
                if xt_nxt is not None:
                    hp.setdefault(b + 2, []).append(emit_pair(b + 2, tt, xt_nxt))
            if xt_nxt is not None:
                emit_halo(hp[b + 1], hp[b + 2])
            elif b + 2 == NB:
                emit_edge_zero(hp[NB - 1], top=False)
            del hp[b]
    nc.finalize()
    return nc


def prep_weights(w_in, w_dw, fft_w, w_out):
    w_in2 = np.asarray(w_in)[:, :, 0, 0].astype(np.float32)        # [510, 192]
    w_dw2 = np.asarray(w_dw)[:, 0].reshape(2 * HID, 9).astype(np.float32)
    wf = np.asarray(fft_w)[:, 0, 0].reshape(HID, 4).astype(np.float32)
    w_out2 = np.asarray(w_out)[:, :, 0, 0].astype(np.float32)      # [192, 510]

    wslot = np.zeros((NSLOT, DIM), np.float32)
    fslot = np.zeros((NSLOT, 4), np.float32)
    wslot[VALID_SLOT] = w_in2[HID_OF_SLOT[VALID_SLOT]]
    fslot[VALID_SLOT] = wf[HID_OF_SLOT[VALID_SLOT]] * 0.25
    wperm = wslot[PAIR_PERM]
    fperm = fslot[PAIR_PERM]
    win4 = np.zeros((2, 96, 4 * NSLOT), np.float32)
    for kc in range(2):
        for k in range(4):
            win4[kc, :, k * NSLOT:(k + 1) * NSLOT] = (
                wperm[:, 96 * kc:96 * (kc + 1)] * fperm[:, k:k + 1]).T

    # dwconv stationary: parts 0-63 = x1 (output o=128a+m reads channel
    # 64a+m//2 at part m//2), parts 64-127 = x2 (output 510+128a+m reads
    # channel 255+64a+m//2 at part 64+m//2).
    dws = np.zeros((128, 36 * 128), np.float32)
    for a in range(4):
        for t in range(9):
            col = (a * 9 + t) * 128
            for m in range(OW[a]):
                dws[m // 2, col + m] = w_dw2[128 * a + m, t]
                dws[64 + m // 2, col + m] = w_dw2[510 + 128 * a + m, t]

    woutT = np.zeros((128, 4 * 192), np.float32)
    for a in range(4):
        kw = OW[a]
        woutT[0:kw, a * 192:(a + 1) * 192] = w_out2[:, 128 * a:128 * a + kw].T
    return (win4.astype(BF16NP), dws.astype(BF16NP), woutT.astype(BF16NP))


def prep_x_planes(x1img):
    """Forward 2x2 Hadamard butterfly on the host -> [DIM, 4*4096] bf16."""
    xr = np.asarray(x1img, np.float32).reshape(DIM, 64, 2, 64, 2)
    q = np.stack([xr[:, :, 0, :, 0], xr[:, :, 0, :, 1],
                  xr[:, :, 1, :, 0], xr[:, :, 1, :, 1]], 1)   # [DIM,4,64,64]
    planes = np.einsum('kq,cqyx->ckyx', H4, q)
    return planes.reshape(DIM, 4 * 4096).astype(BF16NP)


_NC = None


def kernel(x, w_in, w_dw, fft_w, w_out):
    global _NC
    if _NC is None:
        _NC = build_module()
    win4, dws, woutT = prep_weights(w_in, w_dw, fft_w, w_out)
    x = np.ascontiguousarray(np.asarray(x), dtype=np.float32)
    in_maps = [
        {"xp": prep_x_planes(x[i]), "w_in4": win4, "dw_stat": dws, "w_outT": woutT}
        for i in range(NCORES)
    ]
    res = run_bass_kernel_spmd(_NC, in_maps, list(range(NCORES)))
    out = np.stack([res.results[i]["out"].reshape(DIM, H, W) for i in range(NCORES)])
    return out.astype(np.float32)
